# revision 1
# baseline (speedup 1.0000x reference)
"""Trainium2 Bass kernel for multi-head attention (B=4, C=256, N=4096, H=4).

Sharding: 16 (batch, head) pairs across 8 cores -> each core computes 2 heads
of one batch entirely locally (no collectives). The final projection is
column-separable over heads, so each core emits a partial [C, N] output and
the host sums the two partials per batch plus bias/residual terms.

Per-core pipeline (all matmuls bf16, accumulation f32 in PSUM):
  q2/k2 = W[2 heads] @ x          [128, N]  (q gets +bq and *1/sqrt(dk) folded)
  vT    = x^T @ WvT               [N, 128] tiles, with ones columns appended
  S^T   = k2^T q2 per (128 key x 512 query) tile, 2 heads row-packed in PE
  P     = exp(S^T) on ScalarE (no max subtraction; scores ~ N(0,1))
  pv    = [v | 1]^T P  -> attention numerator + denominator row via ones col
  at    = pv[:64] * (1/pv[64])    broadcast via DMA
  out   = WpT @ at  (partial final projection)

Bias folds: bk cancels exactly in softmax (constant along key axis);
bv folds into a host-side constant (attention rows sum to 1); bq on device.
"""

import sys

import numpy as np
import ml_dtypes

if "/opt/trn_rl_repo" not in sys.path:
    sys.path.insert(0, "/opt/trn_rl_repo")

B, C, N, H = 4, 256, 4096, 4
DK = 64
DD = 128          # 2 heads * DK
NB = 512          # query block
NBLK = N // NB    # 8
MT = 128          # key tile
MTILES = N // MT  # 32
MG = 1            # key tiles per PSUM group
NGRP = MTILES // MG
SCALE = 1.0 / np.sqrt(DK)

_NC_CACHE = {}


def build_nc():
    import concourse.bass as bass
    import concourse.mybir as mybir
    import concourse.tile as tile

    bf16 = mybir.dt.bfloat16
    f32 = mybir.dt.float32
    Exp = mybir.ActivationFunctionType.Exp
    Alu = mybir.AluOpType

    nc = bass.Bass(target_bir_lowering=False)

    x_d = nc.declare_dram_parameter("x", [C, N], bf16, isOutput=False)
    wqT_d = nc.declare_dram_parameter("wqT", [C, DD], bf16, isOutput=False)
    wkT_d = nc.declare_dram_parameter("wkT", [C, DD], bf16, isOutput=False)
    wvT_d = nc.declare_dram_parameter("wvT", [C, DD], bf16, isOutput=False)
    wpT_d = nc.declare_dram_parameter("wpT", [2, DD, DD], bf16, isOutput=False)
    bq_d = nc.declare_dram_parameter("bq2", [DD, 1], f32, isOutput=False)
    out_d = nc.declare_dram_parameter("out", [C, N], f32, isOutput=True)

    with tile.TileContext(nc) as tc:
        with (
            tc.tile_pool(name="singles", bufs=1) as singles,
            tc.tile_pool(name="ppool", bufs=3) as ppool,
            tc.tile_pool(name="apool", bufs=3) as apool,
            tc.tile_pool(name="dpool", bufs=2, space="DRAM") as dpool,
            tc.tile_pool(name="psA", bufs=1, space="PSUM") as psA,
            tc.tile_pool(name="psS", bufs=3, space="PSUM") as psS,
            tc.tile_pool(name="psPV", bufs=4, space="PSUM") as psPV,
        ):
            # ---- load inputs ----
            x_sb = singles.tile([128, 2, N], bf16)
            nc.sync.dma_start(
                out=x_sb, in_=x_d[:, :].rearrange("(ko ki) n -> ki ko n", ki=128)
            )
            wq_sb = singles.tile([128, 2, DD], bf16)
            nc.sync.dma_start(
                out=wq_sb, in_=wqT_d[:, :].rearrange("(ko ki) m -> ki ko m", ki=128)
            )
            wk_sb = singles.tile([128, 2, DD], bf16)
            nc.sync.dma_start(
                out=wk_sb, in_=wkT_d[:, :].rearrange("(ko ki) m -> ki ko m", ki=128)
            )
            wv_sb = singles.tile([128, 2, DD], bf16)
            nc.sync.dma_start(
                out=wv_sb, in_=wvT_d[:, :].rearrange("(ko ki) m -> ki ko m", ki=128)
            )
            wp_sb = singles.tile([128, 2, DD], bf16)
            nc.sync.dma_start(
                out=wp_sb, in_=wpT_d[:, :, :].rearrange("j d m -> d j m")
            )
            bq_sb = singles.tile([128, 1], f32)
            nc.sync.dma_start(out=bq_sb, in_=bq_d[:, :])

            # Warm engine vector-clocks on one-time input DMAs so steady-state
            # instructions carry at most one semaphore wait (walrus limit).
            scr = singles.tile([128, 1], f32)
            nc.vector.tensor_copy(out=scr, in_=bq_sb)
            zero_sb = singles.tile([128, 1], f32)
            nc.vector.memset(zero_sb, 0.0)
            scr_out = singles.tile([128, 1], f32)
            nc.scalar.activation(out=scr_out, in_=scr, func=Exp, bias=zero_sb)
            # PE: observe each input-DMA queue once (dummy weight loads)
            nc.tensor.ldweights(x_sb[:, 0, 0:128])
            nc.tensor.ldweights(wq_sb[:, 0, :])
            nc.tensor.ldweights(wk_sb[:, 0, :])
            nc.tensor.ldweights(wv_sb[:, 0, :])
            nc.tensor.ldweights(wp_sb[:, 0, :])

            # ---- projections ----
            # Order: k2 fully, then vte, then q2 per block - attention block
            # nb only needs full k2/vte plus q2[:, nb], so q2 projections of
            # later blocks overlap the first attention blocks.
            q2 = singles.tile([128, N], bf16)
            k2 = singles.tile([128, N], bf16)
            for nb in range(NBLK):
                nsl = slice(nb * NB, (nb + 1) * NB)
                psk = psA.tile([128, NB], f32, tag="psA")
                for ko in range(2):
                    nc.tensor.matmul(
                        psk, wk_sb[:, ko, :], x_sb[:, ko, nsl],
                        start=(ko == 0), stop=(ko == 1), skip_group_check=True,
                    )
                nc.vector.tensor_copy(out=k2[:, nsl], in_=psk)

            # vT with ones columns: [64 v_h0 | 1 | 64 v_h1 | 1]
            vte = singles.tile([128, MTILES, 130], bf16)
            nc.vector.memset(vte[:, :, 64:65], 1.0)
            nc.vector.memset(vte[:, :, 129:130], 1.0)
            for g4 in range(MTILES // 4):
                psv = psA.tile([128, NB], f32, tag="psA")
                for dj in range(4):
                    j = g4 * 4 + dj
                    msl = slice(j * MT, (j + 1) * MT)
                    for ko in range(2):
                        nc.tensor.matmul(
                            psv[:, dj * 128:(dj + 1) * 128],
                            x_sb[:, ko, msl], wv_sb[:, ko, :],
                            start=(ko == 0), stop=(ko == 1), skip_group_check=True,
                        )
                psv3 = psv.rearrange("p (j c) -> p j c", c=128)
                nc.vector.tensor_copy(
                    out=vte[:, g4 * 4:(g4 + 1) * 4, 0:64], in_=psv3[:, :, 0:64]
                )
                nc.vector.tensor_copy(
                    out=vte[:, g4 * 4:(g4 + 1) * 4, 65:129], in_=psv3[:, :, 64:128]
                )

            for nb in range(NBLK):
                nsl = slice(nb * NB, (nb + 1) * NB)
                psq = psA.tile([128, NB], f32, tag="psA")
                for ko in range(2):
                    nc.tensor.matmul(
                        psq, wq_sb[:, ko, :], x_sb[:, ko, nsl],
                        start=(ko == 0), stop=(ko == 1), skip_group_check=True,
                    )
                # q2 = (Wq x + bq) * scale
                nc.vector.tensor_scalar(
                    q2[:, nsl], psq, bq_sb, float(SCALE), Alu.add, Alu.mult
                )

            # PE: observe the DVE tick that finished vte
            nc.tensor.ldweights(vte[:, MTILES - 1, 0:65])

            # ---- attention ----
            import os as _os
            nblk_run = int(_os.environ.get("KERNEL_NBLK", NBLK))
            at2_prev = None
            for nb in range(nblk_run):
                nsl = slice(nb * NB, (nb + 1) * NB)
                if at2_prev is not None:
                    # PE: observe DVE's at2 tick of the previous block so the
                    # first PV matmul below carries only the ACT wait
                    nc.tensor.ldweights(at2_prev[:, 0:128])
                pv0 = psPV.tile([128, NB], f32, tag="pv")
                pv1 = psPV.tile([128, NB], f32, tag="pv")
                for jg in range(NGRP):
                    ps0 = psS.tile([128, MG * NB], f32, tag="s")
                    ps1 = psS.tile([128, MG * NB], f32, tag="s")
                    for dj in range(MG):
                        j = jg * MG + dj
                        msl = slice(j * MT, (j + 1) * MT)
                        dsl = slice(dj * NB, (dj + 1) * NB)
                        nc.tensor.matmul(
                            ps0[:, dsl], k2[0:64, msl], q2[0:64, nsl],
                            start=True, stop=True, tile_position=(0, 0),
                            skip_group_check=True,
                        )
                        nc.tensor.matmul(
                            ps1[:, dsl], k2[64:128, msl], q2[64:128, nsl],
                            start=True, stop=True, tile_position=(64, 0),
                            skip_group_check=True,
                        )
                    pt0 = ppool.tile([128, MG * NB], bf16, tag="pt")
                    nc.scalar.activation(pt0, ps0, Exp, bias=zero_sb)
                    pt1 = ppool.tile([128, MG * NB], bf16, tag="pt")
                    if jg % 4 == 3:
                        # exp via Schraudolph bitcast: bf16 bits =
                        # round(s*128*log2e + (127<<7) - c); offloads ACT
                        nc.vector.tensor_scalar(
                            pt1.bitcast(mybir.dt.int16), ps1,
                            184.6650085511249, 16250.4, Alu.mult, Alu.add
                        )
                    else:
                        nc.scalar.activation(pt1, ps1, Exp, bias=zero_sb)
                    for dj in range(MG):
                        j = jg * MG + dj
                        dsl = slice(dj * NB, (dj + 1) * NB)
                        first = jg == 0 and dj == 0
                        last = jg == NGRP - 1 and dj == MG - 1
                        nc.tensor.matmul(
                            pv0[0:65, :], vte[:, j, 0:65], pt0[:, dsl],
                            start=first, stop=last, skip_group_check=True,
                        )
                        nc.tensor.matmul(
                            pv1[0:65, :], vte[:, j, 65:130], pt1[:, dsl],
                            start=first, stop=last, skip_group_check=True,
                        )

                # softmax denominators -> divide
                den = apool.tile([128, 2 * NB], f32, tag="den")
                nc.vector.reciprocal(out=den[64:65, 0:NB], in_=pv0[64:65, :])
                nc.vector.reciprocal(out=den[64:65, NB:2 * NB], in_=pv1[64:65, :])
                dd = dpool.tile([2, NB], f32, tag="dd")
                nc.gpsimd.dma_start(out=dd[0:1, :], in_=den[64:65, 0:NB])
                nc.gpsimd.dma_start(out=dd[1:2, :], in_=den[64:65, NB:2 * NB])
                bc = apool.tile([128, NB], f32, tag="bc")
                nc.gpsimd.dma_start(
                    out=bc[0:64, :], in_=dd[0:1, :].to_broadcast((64, NB))
                )
                nc.gpsimd.dma_start(
                    out=bc[64:128, :], in_=dd[1:2, :].to_broadcast((64, NB))
                )
                at2 = apool.tile([128, NB], bf16, tag="at")
                nc.vector.tensor_tensor(at2[0:64, :], pv0[0:64, :], bc[0:64, :], Alu.mult)
                nc.vector.tensor_tensor(at2[64:128, :], pv1[0:64, :], bc[64:128, :], Alu.mult)

                # partial final projection
                for oh in range(2):
                    pso = psA.tile([128, NB], f32, tag="psA")
                    nc.tensor.matmul(
                        pso, wp_sb[:, oh, :], at2,
                        start=True, stop=True, skip_group_check=True,
                    )
                    osb = apool.tile([128, NB], f32, tag="osb")
                    nc.scalar.activation(
                        out=osb, in_=pso, func=mybir.ActivationFunctionType.Copy
                    )
                    nc.gpsimd.dma_start(
                        out=out_d[:, :][oh * 128:(oh + 1) * 128, nsl], in_=osb
                    )
                at2_prev = at2
    return nc


def split_multiwaits(nc):
    """The staged walrus accepts at most one sync-wait per instruction; Tile
    emits several. Hoist all but one wait onto same-engine NOPs placed just
    before the instruction (engine program order makes this equivalent)."""
    import concourse.mybir as mybir

    n = 0
    for fn in nc.m.functions:
        for blk in fn.blocks:
            new = []
            for inst in blk.instructions:
                si = getattr(inst, "sync_info", None)
                waits = list(si.on_wait) if si is not None and si.on_wait else []
                if len(waits) > 1:
                    for k, w in enumerate(waits[:-1]):
                        new.append(mybir.InstNoOp(
                            name=f"{inst.name}-w{k}",
                            engine=inst.engine,
                            ins=[], outs=[],
                            sync_info=mybir.SyncInfo(on_wait=[w], on_update=[]),
                        ))
                        n += 1
                    inst.sync_info = mybir.SyncInfo(
                        on_wait=[waits[-1]], on_update=list(si.on_update)
                    )
                new.append(inst)
            blk.instructions = new
    return n


def _get_nc():
    if "nc" not in _NC_CACHE:
        nc = build_nc()
        split_multiwaits(nc)
        _NC_CACHE["nc"] = nc
    return _NC_CACHE["nc"]


def _make_in_maps(x, wq, bq, wk, wv, wp):
    bf = ml_dtypes.bfloat16
    in_maps = []
    for core in range(8):
        b = core // 2
        hp = core % 2
        rs = slice(hp * DD, (hp + 1) * DD)
        in_maps.append({
            "x": np.ascontiguousarray(x[b]).astype(bf),
            "wqT": np.ascontiguousarray(wq[rs, :].T).astype(bf),
            "wkT": np.ascontiguousarray(wk[rs, :].T).astype(bf),
            "wvT": np.ascontiguousarray(wv[rs, :].T).astype(bf),
            "wpT": np.stack(
                [np.ascontiguousarray(wp[j * DD:(j + 1) * DD, rs].T) for j in range(2)]
            ).astype(bf),
            "bq2": np.ascontiguousarray(bq[rs]).reshape(DD, 1).astype(np.float32),
        })
    return in_maps


def run(x, wq, bq, wk, bk, wv, bv, wp, bp, trace=False):
    from concourse.bass_utils import run_bass_kernel_spmd

    x = np.asarray(x, dtype=np.float32)
    wq = np.asarray(wq, dtype=np.float32)
    bq = np.asarray(bq, dtype=np.float32)
    wk = np.asarray(wk, dtype=np.float32)
    wv = np.asarray(wv, dtype=np.float32)
    bv = np.asarray(bv, dtype=np.float32)
    wp = np.asarray(wp, dtype=np.float32)
    bp = np.asarray(bp, dtype=np.float32)

    nc = _get_nc()
    in_maps = _make_in_maps(x, wq, bq, wk, wv, wp)
    res = run_bass_kernel_spmd(nc, in_maps, core_ids=list(range(8)), trace=trace)
    parts = [r["out"].astype(np.float32) for r in res.results]

    const = (bp + wp @ bv).astype(np.float32)[:, None]  # [C, 1]
    out = np.empty((B, C, N), dtype=np.float32)
    for b in range(B):
        out[b] = parts[2 * b] + parts[2 * b + 1] + x[b] + const
    return out, res


def kernel(**inputs):
    out, _ = run(**inputs)
    return out



# revision 4
# speedup vs baseline: 1.1208x; 1.1208x over previous
"""Trainium2 Bass kernel for multi-head attention (B=4, C=256, N=4096, H=4).

Sharding: 8 cores = (batch b, query-half) pairs. Each core computes ALL 4
heads for 2048 queries of one batch, attending over all 4096 keys, and
emits a COMPLETE fp16 [C, 2048] output slice (residual + all bias terms
folded on device) -- outputs are disjoint, no host-side reduction.

Key-order trick: cores with half=1 receive x with the two N-halves swapped
so their queries are the first 2048 columns (softmax is invariant to key
permutation; K and V share the same order).

Per-core pipeline (matmuls bf16, f32 PSUM):
  k4  = Wk x                      [128, 2hp, 4096]
  vte = x^T WvT with ones col     [128, 4h, 32j, 65]   (v | 1 per head)
  q4  = (Wq x + bq) * scale       [128, 2hp, 2048]
  per (query block qb, head-pair hp), software-pipelined over key tiles:
    S^T = k4^T q4 (2 subheads row-tiled concurrently in PE)
    P   = exp(S^T)   split ~55/45 between ACT (table exp) and DVE
                     (Schraudolph bitcast exp), PV lags one tile behind S
    pv  = [v | 1]^T P  (numerator + denominator via ones column)
    bc  = ones64 (x) 1/den    K=1 PE outer-product broadcast (no DMA trip)
    at  = pv * bc
  out = Wp at + (x + bp + Wp bv)  -> fp16 DMA out

Bias folds: bk cancels in softmax; bv enters via wp@bv in the epilogue
constant; bq on device.
"""

import os
import sys

import numpy as np
import ml_dtypes

if "/opt/trn_rl_repo" not in sys.path:
    sys.path.insert(0, "/opt/trn_rl_repo")

B, C, N, H = 4, 256, 4096, 4
DK = 64
NQ = 2048         # queries per core
NB = 512          # query block
QBLK = NQ // NB   # 4
MT = 128          # key tile
MTILES = N // MT  # 32
SCALE = 1.0 / np.sqrt(DK)

# of the 32 key tiles' second-subhead exp's, how many go to ACT (rest DVE)
ACT_EXTRA = int(os.environ.get("KERNEL_ACT_EXTRA", 5))

_NC_CACHE = {}


def build_nc():
    import concourse.bass as bass
    import concourse.mybir as mybir
    import concourse.tile as tile

    bf16 = mybir.dt.bfloat16
    f16 = mybir.dt.float16
    f32 = mybir.dt.float32
    Exp = mybir.ActivationFunctionType.Exp
    Alu = mybir.AluOpType

    nc = bass.Bass(target_bir_lowering=False)

    x_d = nc.declare_dram_parameter("x", [C, N], bf16, isOutput=False)
    wqT_d = nc.declare_dram_parameter("wqT", [C, C], bf16, isOutput=False)
    wkT_d = nc.declare_dram_parameter("wkT", [C, C], bf16, isOutput=False)
    wvT_d = nc.declare_dram_parameter("wvT", [C, C], bf16, isOutput=False)
    wpT_d = nc.declare_dram_parameter("wpT", [C, C], bf16, isOutput=False)
    bq_d = nc.declare_dram_parameter("bq2", [128, 2], f32, isOutput=False)
    cadd_d = nc.declare_dram_parameter("cadd2", [128, 2], f32, isOutput=False)
    out_d = nc.declare_dram_parameter("out", [C, NQ], f16, isOutput=True)

    with tile.TileContext(nc) as tc:
        with (
            tc.tile_pool(name="singles", bufs=1) as singles,
            tc.tile_pool(name="ppool", bufs=4) as ppool,
            tc.tile_pool(name="apool", bufs=4) as apool,
            tc.tile_pool(name="opool", bufs=2) as opool,
            tc.tile_pool(name="dpool", bufs=2, space="DRAM") as dpool,
            tc.tile_pool(name="psA", bufs=1, space="PSUM") as psA,
            tc.tile_pool(name="psS", bufs=3, space="PSUM") as psS,
            tc.tile_pool(name="psPV", bufs=4, space="PSUM") as psPV,
        ):
            # ---- load inputs ----
            # x in 4 chunks so the k4 projections can start early
            x_sb = singles.tile([128, 2, N], bf16)
            for xc_i in range(4):
                xsl = slice(xc_i * (N // 4), (xc_i + 1) * (N // 4))
                nc.sync.dma_start(
                    out=x_sb[:, :, xsl],
                    in_=x_d[:, xsl].rearrange("(ko ki) n -> ki ko n", ki=128),
                )
            wq_sb = singles.tile([128, 2, C], bf16)
            nc.sync.dma_start(
                out=wq_sb, in_=wqT_d[:, :].rearrange("(ko ki) m -> ki ko m", ki=128)
            )
            wk_sb = singles.tile([128, 2, C], bf16)
            nc.sync.dma_start(
                out=wk_sb, in_=wkT_d[:, :].rearrange("(ko ki) m -> ki ko m", ki=128)
            )
            wv_sb = singles.tile([128, 2, C], bf16)
            nc.sync.dma_start(
                out=wv_sb, in_=wvT_d[:, :].rearrange("(ko ki) m -> ki ko m", ki=128)
            )
            wp_sb = singles.tile([128, 2, C], bf16)
            nc.sync.dma_start(
                out=wp_sb, in_=wpT_d[:, :].rearrange("(ko ki) m -> ki ko m", ki=128)
            )
            bq_sb = singles.tile([128, 2], f32)
            nc.sync.dma_start(out=bq_sb, in_=bq_d[:, :])
            cadd_sb = singles.tile([128, 2], f32)
            nc.sync.dma_start(out=cadd_sb, in_=cadd_d[:, :])

            # Warm engine vector-clocks on one-time input DMAs so steady-state
            # instructions carry at most one semaphore wait (walrus limit).
            scr = singles.tile([128, 2], f32)
            nc.vector.tensor_copy(out=scr, in_=bq_sb)
            scr2 = singles.tile([128, 2], f32)
            nc.vector.tensor_copy(out=scr2, in_=cadd_sb)
            zero_sb = singles.tile([128, 1], f32)
            nc.vector.memset(zero_sb, 0.0)
            scr_out = singles.tile([128, 2], f32)
            nc.scalar.activation(out=scr_out, in_=scr, func=Exp, bias=zero_sb)
            nc.tensor.ldweights(x_sb[:, 0, 0:128])
            nc.tensor.ldweights(wq_sb[:, 0, 0:128])
            nc.tensor.ldweights(wk_sb[:, 0, 0:128])
            nc.tensor.ldweights(wv_sb[:, 0, 0:128])
            nc.tensor.ldweights(wp_sb[:, 0, 0:128])

            # ---- projections ----
            # k4: keys for all 4 heads (2 head-pairs x 64-row subheads)
            k4 = singles.tile([128, 2, N], bf16)
            for hp in range(2):
                csl = slice(hp * 128, (hp + 1) * 128)
                for nb in range(N // NB):
                    nsl = slice(nb * NB, (nb + 1) * NB)
                    psk = psS.tile([128, NB], f32, tag="s")
                    for ko in range(2):
                        nc.tensor.matmul(
                            psk, wk_sb[:, ko, csl], x_sb[:, ko, nsl],
                            start=(ko == 0), stop=(ko == 1), skip_group_check=True,
                        )
                    nc.vector.tensor_copy(out=k4[:, hp, nsl], in_=psk)

            # vte: v^T tiles with ones column per head: [.., h, j, 0:64]=v,
            # [.., h, j, 64]=1
            vte = singles.tile([128, H, MTILES, 65], bf16)
            for h in range(H):
                nc.vector.memset(vte[:, h, :, 64:65], 1.0)
            for g in range(MTILES // 2):
                psv = psS.tile([128, NB], f32, tag="s")
                for t in range(2):
                    j = g * 2 + t
                    msl = slice(j * MT, (j + 1) * MT)
                    for ko in range(2):
                        nc.tensor.matmul(
                            psv[:, t * 256:(t + 1) * 256],
                            x_sb[:, ko, msl], wv_sb[:, ko, :],
                            start=(ko == 0), stop=(ko == 1), skip_group_check=True,
                        )
                psv3 = psv.rearrange("p (t c) -> p t c", c=256)
                for h in range(H):
                    nc.vector.tensor_copy(
                        out=vte[:, h, g * 2:g * 2 + 2, 0:64],
                        in_=psv3[:, :, h * 64:(h + 1) * 64],
                    )

            # q4: queries for all 4 heads, this core's 2048 queries,
            # bias + scale folded
            q4 = singles.tile([128, 2, NQ], bf16)
            for hp in range(2):
                csl = slice(hp * 128, (hp + 1) * 128)
                for qb in range(QBLK):
                    qsl = slice(qb * NB, (qb + 1) * NB)
                    psq = psS.tile([128, NB], f32, tag="s")
                    for ko in range(2):
                        nc.tensor.matmul(
                            psq, wq_sb[:, ko, csl], x_sb[:, ko, qsl],
                            start=(ko == 0), stop=(ko == 1), skip_group_check=True,
                        )
                    nc.vector.tensor_scalar(
                        q4[:, hp, qsl], psq, bq_sb[:, hp:hp + 1], float(SCALE),
                        Alu.add, Alu.mult,
                    )

            # xc: residual + epilogue constant, fp32 (on the idle GpSimd)
            xc = singles.tile([128, 2, NQ], f32)
            for oh in range(2):
                nc.gpsimd.tensor_scalar_add(
                    xc[:, oh, :], x_sb[:, oh, 0:NQ], cadd_sb[:, oh:oh + 1]
                )

            # PE: observe the DVE ticks that finished vte / q4
            nc.tensor.ldweights(vte[:, H - 1, MTILES - 1, 0:65])

            # ---- attention ----
            # Per (query block, head-pair) segment: S^T -> exp -> PV, with PV
            # lagging one key tile behind S (PE never waits on exp). The
            # normalize/project chain of a finished segment (bc outer-product
            # + final projection) is DEFERRED into the next segment's PE
            # stream so the PE FIFO never blocks on recip/at4 latency.
            qblk_run = int(os.environ.get("KERNEL_NBLK", QBLK))
            pending = []  # deferred emitters for end-of-segment PE ops
            at_pair = {}  # hp -> at4 tile of current qb

            def make_norm(qb, hp, pv0, pv1, bc_sb):
                def emit():
                    at4 = apool.tile([128, NB], bf16, tag="at")
                    nc.vector.tensor_tensor(
                        at4[0:64, :], pv0[0:64, :], bc_sb[0:64, :], Alu.mult
                    )
                    nc.vector.tensor_tensor(
                        at4[64:128, :], pv1[0:64, :], bc_sb[64:128, :], Alu.mult
                    )
                    at_pair[hp] = at4
                return emit

            def make_epilogue(qb):
                def emit():
                    qsl = slice(qb * NB, (qb + 1) * NB)
                    ats = (at_pair[0], at_pair[1])
                    for oh in range(2):
                        osl = slice(oh * 128, (oh + 1) * 128)
                        pso = psA.tile([128, NB], f32, tag="psA")
                        for hp in range(2):
                            nc.tensor.matmul(
                                pso, wp_sb[:, hp, osl], ats[hp],
                                start=(hp == 0), stop=(hp == 1),
                                skip_group_check=True,
                            )
                        osb = opool.tile([128, NB], f16, tag="osb")
                        nc.vector.tensor_tensor(osb, pso, xc[:, oh, qsl], Alu.add)
                        nc.gpsimd.dma_start(out=out_d[:, :][osl, qsl], in_=osb)
                return emit

            for qb in range(qblk_run):
                qsl = slice(qb * NB, (qb + 1) * NB)
                for hp in range(2):
                    pv0 = psPV.tile([128, NB], f32, tag="pv")
                    pv1 = psPV.tile([128, NB], f32, tag="pv")
                    pend = None
                    for jg in range(MTILES):
                        msl = slice(jg * MT, (jg + 1) * MT)
                        ps0 = psS.tile([128, NB], f32, tag="s")
                        ps1 = psS.tile([128, NB], f32, tag="s")
                        nc.tensor.matmul(
                            ps0, k4[0:64, hp, msl], q4[0:64, hp, qsl],
                            start=True, stop=True, tile_position=(0, 0),
                            skip_group_check=True,
                        )
                        nc.tensor.matmul(
                            ps1, k4[64:128, hp, msl], q4[64:128, hp, qsl],
                            start=True, stop=True, tile_position=(64, 0),
                            skip_group_check=True,
                        )
                        pt0 = ppool.tile([128, NB], bf16, tag="pt")
                        nc.scalar.activation(pt0, ps0, Exp, bias=zero_sb)
                        pt1 = ppool.tile([128, NB], bf16, tag="pt")
                        if jg < ACT_EXTRA:
                            nc.scalar.activation(pt1, ps1, Exp, bias=zero_sb)
                        else:
                            # exp via Schraudolph bitcast: bf16 bits =
                            # round(s*128*log2e + (127<<7) - c); offloads ACT
                            nc.vector.tensor_scalar(
                                pt1.bitcast(mybir.dt.int16), ps1,
                                184.6650085511249, 16250.4, Alu.mult, Alu.add,
                            )
                        if jg == 1 and pending:
                            for f in pending:
                                f()
                            pending = []
                        # software-pipeline: PV for tile jg-1 issues behind
                        # S/exp of tile jg so PE never waits on the exp
                        if pend is not None:
                            pj, ppt0, ppt1 = pend
                            nc.tensor.matmul(
                                pv0[0:65, :], vte[:, 2 * hp, pj, :], ppt0,
                                start=(pj == 0), stop=(pj == MTILES - 1),
                                skip_group_check=True,
                            )
                            nc.tensor.matmul(
                                pv1[0:65, :], vte[:, 2 * hp + 1, pj, :], ppt1,
                                start=(pj == 0), stop=(pj == MTILES - 1),
                                skip_group_check=True,
                            )
                        pend = (jg, pt0, pt1)
                    pj, ppt0, ppt1 = pend
                    nc.tensor.matmul(
                        pv0[0:65, :], vte[:, 2 * hp, pj, :], ppt0,
                        start=(pj == 0), stop=(pj == MTILES - 1),
                        skip_group_check=True,
                    )
                    nc.tensor.matmul(
                        pv1[0:65, :], vte[:, 2 * hp + 1, pj, :], ppt1,
                        start=(pj == 0), stop=(pj == MTILES - 1),
                        skip_group_check=True,
                    )

                    # softmax denominators: reciprocal -> DMA broadcast into
                    # SBUF (round-trip latency hidden by the deferral)
                    den = apool.tile([1, 2 * NB], bf16, tag="den")
                    with nc.allow_low_precision(reason="bf16 1/den broadcast"):
                        nc.vector.reciprocal(out=den[0:1, 0:NB], in_=pv0[64:65, :])
                        nc.vector.reciprocal(out=den[0:1, NB:2 * NB], in_=pv1[64:65, :])
                    dd = dpool.tile([1, 2 * NB], bf16, tag="dd")
                    nc.gpsimd.dma_start(out=dd, in_=den[0:1, :])
                    bc_sb = apool.tile([128, NB], bf16, tag="bc")
                    nc.gpsimd.dma_start(
                        out=bc_sb[0:64, :], in_=dd[0:1, 0:NB].to_broadcast((64, NB))
                    )
                    nc.gpsimd.dma_start(
                        out=bc_sb[64:128, :],
                        in_=dd[0:1, NB:2 * NB].to_broadcast((64, NB)),
                    )
                    pending.append(make_norm(qb, hp, pv0, pv1, bc_sb))
                    if hp == 1:
                        pending.append(make_epilogue(qb))
            for f in pending:
                f()
            pending = []
    return nc


def split_multiwaits(nc):
    """The staged walrus accepts at most one sync-wait per instruction; Tile
    emits several. Hoist all but one wait onto same-engine NOPs placed just
    before the instruction (engine program order makes this equivalent)."""
    import concourse.mybir as mybir

    n = 0
    for fn in nc.m.functions:
        for blk in fn.blocks:
            new = []
            for inst in blk.instructions:
                si = getattr(inst, "sync_info", None)
                waits = list(si.on_wait) if si is not None and si.on_wait else []
                if len(waits) > 1:
                    for k, w in enumerate(waits[:-1]):
                        new.append(mybir.InstNoOp(
                            name=f"{inst.name}-w{k}",
                            engine=inst.engine,
                            ins=[], outs=[],
                            sync_info=mybir.SyncInfo(on_wait=[w], on_update=[]),
                        ))
                        n += 1
                    inst.sync_info = mybir.SyncInfo(
                        on_wait=[waits[-1]], on_update=list(si.on_update)
                    )
                new.append(inst)
            blk.instructions = new
    return n


def _get_nc():
    if "nc" not in _NC_CACHE:
        nc = build_nc()
        split_multiwaits(nc)
        _NC_CACHE["nc"] = nc
    return _NC_CACHE["nc"]


def _make_in_maps(x, wq, bq, wk, wv, wp, bv, bp):
    bf = ml_dtypes.bfloat16
    wqT = np.ascontiguousarray(wq.T).astype(bf)
    wkT = np.ascontiguousarray(wk.T).astype(bf)
    wvT = np.ascontiguousarray(wv.T).astype(bf)
    wpT = np.ascontiguousarray(wp.T).astype(bf)
    bq2 = np.ascontiguousarray(bq.reshape(2, 128).T).astype(np.float32)
    cadd2 = np.ascontiguousarray(
        (bp + wp @ bv).reshape(2, 128).T
    ).astype(np.float32)
    in_maps = []
    for core in range(8):
        b = core // 2
        half = core % 2
        if half == 0:
            xp = x[b]
        else:
            xp = np.concatenate([x[b][:, NQ:], x[b][:, :NQ]], axis=1)
        in_maps.append({
            "x": np.ascontiguousarray(xp).astype(bf),
            "wqT": wqT,
            "wkT": wkT,
            "wvT": wvT,
            "wpT": wpT,
            "bq2": bq2,
            "cadd2": cadd2,
        })
    return in_maps


def run(x, wq, bq, wk, bk, wv, bv, wp, bp, trace=False):
    from concourse.bass_utils import run_bass_kernel_spmd

    x = np.asarray(x, dtype=np.float32)
    wq = np.asarray(wq, dtype=np.float32)
    bq = np.asarray(bq, dtype=np.float32)
    wk = np.asarray(wk, dtype=np.float32)
    wv = np.asarray(wv, dtype=np.float32)
    bv = np.asarray(bv, dtype=np.float32)
    wp = np.asarray(wp, dtype=np.float32)
    bp = np.asarray(bp, dtype=np.float32)

    nc = _get_nc()
    in_maps = _make_in_maps(x, wq, bq, wk, wv, wp, bv, bp)
    res = run_bass_kernel_spmd(nc, in_maps, core_ids=list(range(8)), trace=trace)

    out = np.empty((B, C, N), dtype=np.float32)
    for core in range(8):
        b = core // 2
        half = core % 2
        qsl = slice(half * NQ, (half + 1) * NQ)
        out[b][:, qsl] = res.results[core]["out"].astype(np.float32)
    return out, res


def kernel(**inputs):
    out, _ = run(**inputs)
    return out
